# revision 16
# baseline (speedup 1.0000x reference)
"""Trainium2 Bass kernel for segment_reduce MLP (nn_HeadSemantic_35983236006251).

Math shortcut: since Linear commutes with segment_sum,
    pooled = segment_sum(x @ W_in + b_in) = segment_sum(x) @ W_in + counts * b_in
so the big [N,D]x[D,D] matmul collapses to a [4096,D]x[D,D] one and the kernel
becomes memory-bound streaming of x (1 GB) into per-segment sums.

Sharding: the 4096 segments are split into 32 windows of 128 segments. Window
boundaries in the (sorted) batch vector are found with searchsorted; each of the
8 cores owns 4 consecutive windows and consumes only the x rows overlapping its
windows, so per-segment partials are exact and no cross-core reduction is needed.

Device per 128-row tile: a one-hot matrix S[p,j] = (batch[p]-128w == j) is built
with one DVE compare against an iota constant; one PE matmul S^T @ [x|1]
accumulates [128 seg, 256 feat + count] into PSUM. The tiny MLP runs per core on
its 512 segments in transposed orientation (weights natively serve as lhsT).

Raw-bass implementation (explicit semaphores): this toolchain limits every
engine instruction to ONE attached sync wait, so multi-dependency points use
standalone wait_ge sequencer instructions instead of Tile's packed waits.
"""

import sys
import numpy as np
from contextlib import ExitStack

sys.path.insert(0, "/opt/trn_rl_repo")

import concourse.bass as bass
from concourse import mybir
from concourse.bass_utils import run_bass_kernel_spmd

N = 1_000_000
D = 256
NSEG = 4096
WIN = 128                  # segments per window
N_CORES = 8
NW = (NSEG // WIN) // N_CORES   # windows per core = 4
SEG = NW * WIN                  # segments per core = 512
F32 = mybir.dt.float32
F32R = mybir.dt.float32r
EQ = mybir.AluOpType.is_equal
XS = 24                    # x-tile ring slots
SS = 24                    # one-hot ring slots


def build_program(T):
    """Bass program, identical on all 8 cores (SPMD); T = x tiles per window."""
    nc = bass.Bass()

    x_in = [nc.declare_dram_parameter(f"x{w}", [T * 128, D], F32, False)
            for w in range(NW)]
    ba_in = [nc.declare_dram_parameter(f"ba{w}", [128, T], F32, False)
             for w in range(NW)]
    ones_in = nc.declare_dram_parameter("ones", [128, 2], F32, False)
    onesrow_in = nc.declare_dram_parameter("onesrow", [1, SEG], F32, False)
    iota_in = nc.declare_dram_parameter("iota", [128, 128], F32, False)
    ident_in = nc.declare_dram_parameter("ident", [128, 128], F32, False)
    win_in = nc.declare_dram_parameter("win", [D, D], F32, False)
    bin_in = nc.declare_dram_parameter("bin", [1, D], F32, False)
    w1_in = nc.declare_dram_parameter("w1", [D, 2 * D], F32, False)
    b1_in = nc.declare_dram_parameter("b1", [1, 2 * D], F32, False)
    w2_in = nc.declare_dram_parameter("w2", [2 * D, D], F32, False)
    b2_in = nc.declare_dram_parameter("b2", [1, D], F32, False)
    outT_ext = nc.declare_dram_parameter("outT", [D, SEG], F32, True)

    GT = NW * T   # total x tiles per core

    with ExitStack() as es:
        def sem(name):
            return es.enter_context(nc.semaphore(name))

        def sb(name, shape, dt):
            return es.enter_context(nc.sbuf_tensor(name, shape, dt))

        def psum(name, shape, dt):
            return es.enter_context(nc.psum_tensor(name, shape, dt))

        s_c, s_ba, s_s, s_mm = sem("c"), sem("ba"), sem("s"), sem("mm")
        s_fl, s_cp, s_tr, s_crc = sem("fl"), sem("cp"), sem("tr"), sem("crc")
        s_ptc, s_z, s_zc, s_h = sem("ptc"), sem("z"), sem("zc"), sem("h")
        s_hc, s_o, s_oc, s_do = sem("hc"), sem("o"), sem("oc"), sem("do")
        s_x = [sem(f"x{i}") for i in range(XS)]

        iota_sb = sb("iota_sb", [128, 128], F32)
        ident_sb = sb("ident_sb", [128, 128], F32)
        MDT = F32  # plain-fp32 MLP: fp32r rounding there cost 2.6e-4 rel err
        wink = [sb(f"wink{k}", [128, D], MDT) for k in range(2)]
        bin_sb = sb("bin_sb", [1, D], MDT)
        w1k = [sb(f"w1k{k}", [128, 2 * D], MDT) for k in range(2)]
        b1_sb = sb("b1_sb", [1, 2 * D], MDT)
        w2k = [sb(f"w2k{k}", [128, D], MDT) for k in range(4)]
        b2_sb = sb("b2_sb", [1, D], MDT)
        ba_sb = [sb(f"ba_sb{w}", [128, T], F32) for w in range(NW)]
        xbuf = [sb(f"xb{i}", [128, D + 2], F32R) for i in range(XS)]
        S_sb = [sb(f"S{i}", [128, 128], F32R) for i in range(SS)]
        po = [sb(f"po{w}", [128, D + 1], F32) for w in range(NW)]
        pT = [sb(f"pT{k}", [128, SEG], MDT) for k in range(2)]
        c_row = sb("c_row", [1, SEG], MDT)
        zT = [sb(f"zT{j}", [128, SEG], MDT) for j in range(2)]
        hT = [sb(f"hT{j}", [128, SEG], MDT) for j in range(4)]
        ot = [sb(f"ot{j}", [128, SEG], F32) for j in range(2)]
        ones_row = sb("ones_row", [1, SEG], MDT)

        pb = [psum("pb0", [128, 512], F32), psum("pb1", [128, 512], F32)]
        trA = psum("trA", [128, 512], F32)
        trB = psum("trB", [128, 512], F32)

        NCDMA = 14 + XS  # const DMAs incl. ones cols

        with nc.Block() as block:

            @block.sync
            def _(sp):
                # constants
                sp.dma_start(out=iota_sb[:, :], in_=iota_in[:, :]).then_inc(s_c, 16)
                sp.dma_start(out=ident_sb[:, :], in_=ident_in[:, :]).then_inc(s_c, 16)
                for k in range(2):
                    sp.dma_start(out=wink[k][:, :],
                                 in_=win_in[k * 128:(k + 1) * 128, :].bitcast(MDT)
                                 ).then_inc(s_c, 16)
                sp.dma_start(out=bin_sb[:, :], in_=bin_in[:, :].bitcast(MDT)
                             ).then_inc(s_c, 16)
                for k in range(2):
                    sp.dma_start(out=w1k[k][:, :],
                                 in_=w1_in[k * 128:(k + 1) * 128, :].bitcast(MDT)
                                 ).then_inc(s_c, 16)
                sp.dma_start(out=b1_sb[:, :], in_=b1_in[:, :].bitcast(MDT)
                             ).then_inc(s_c, 16)
                for k in range(4):
                    sp.dma_start(out=w2k[k][:, :],
                                 in_=w2_in[k * 128:(k + 1) * 128, :].bitcast(MDT)
                                 ).then_inc(s_c, 16)
                sp.dma_start(out=b2_sb[:, :], in_=b2_in[:, :].bitcast(MDT)
                             ).then_inc(s_c, 16)
                sp.dma_start(out=ones_row[:, :], in_=onesrow_in[:, :].bitcast(MDT)
                             ).then_inc(s_c, 16)
                for i in range(XS):
                    sp.dma_start(out=xbuf[i][:, D:D + 2],
                                 in_=ones_in[:, :].bitcast(F32R)).then_inc(s_c, 16)
                for w in range(NW):
                    sp.dma_start(out=ba_sb[w][:, :], in_=ba_in[w][:, :]
                                 ).then_inc(s_ba, 16)
                # x stream with ring-slot recycling
                for g in range(GT):
                    w, t = divmod(g, T)
                    if g >= XS:
                        sp.wait_ge(s_mm, g - XS + 1)
                    sp.dma_start(out=xbuf[g % XS][:, 0:D],
                                 in_=x_in[w][t * 128:(t + 1) * 128, :].bitcast(F32R)
                                 ).then_inc(s_x[g % XS], 16)
                # output
                sp.wait_ge(s_oc, 1)
                sp.dma_start(out=outT_ext[0:128, :], in_=ot[0][:, :]).then_inc(s_do, 16)
                sp.wait_ge(s_oc, 2)
                sp.dma_start(out=outT_ext[128:256, :], in_=ot[1][:, :]).then_inc(s_do, 16)
                sp.wait_ge(s_do, 32)

            @block.vector
            def _(v):
                v.wait_ge(s_c, 16 * NCDMA)
                v.wait_ge(s_ba, 16 * NW)
                for g in range(GT):
                    w, t = divmod(g, T)
                    if g >= SS:
                        v.wait_ge(s_mm, g - SS + 1)
                    v.tensor_scalar(S_sb[g % SS][:, :], iota_sb[:, :],
                                    ba_sb[w][:, t:t + 1], None, EQ).then_inc(s_s, 1)
                    if t == T - 1:
                        v.wait_ge(s_mm, (w + 1) * T)
                        v.tensor_copy(po[w][:, :], pb[w % 2][:, 0:D + 1]
                                      ).then_inc(s_fl, 1)
                # ---- MLP copies ----
                v.wait_ge(s_cp, NW)
                v.tensor_copy(c_row[:, :], trA[0:1, 0:SEG]).then_inc(s_crc, 1)
                v.wait_ge(s_tr, 4)
                v.tensor_copy(pT[0][:, 0:128], trB[:, 0:128]).then_inc(s_ptc, 1)
                v.tensor_copy(pT[1][:, 0:128], trB[:, 128:256]).then_inc(s_ptc, 1)
                v.tensor_copy(pT[0][:, 128:256], trB[:, 256:384]).then_inc(s_ptc, 1)
                v.tensor_copy(pT[1][:, 128:256], trB[:, 384:512]).then_inc(s_ptc, 1)
                v.wait_ge(s_tr, 8)
                v.tensor_copy(pT[0][:, 256:384], pb[0][:, 0:128]).then_inc(s_ptc, 1)
                v.tensor_copy(pT[1][:, 256:384], pb[0][:, 128:256]).then_inc(s_ptc, 1)
                v.tensor_copy(pT[0][:, 384:512], pb[0][:, 256:384]).then_inc(s_ptc, 1)
                v.tensor_copy(pT[1][:, 384:512], pb[0][:, 384:512]).then_inc(s_ptc, 1)
                v.wait_ge(s_z, 1)
                v.tensor_copy(zT[0][:, :], trA[:, :]).then_inc(s_zc, 1)
                v.wait_ge(s_z, 2)
                v.tensor_copy(zT[1][:, :], pb[1][:, :]).then_inc(s_zc, 1)
                for j in range(4):
                    v.wait_ge(s_h, j + 1)
                    v.tensor_relu(hT[j][:, :], (trB if j % 2 == 0 else pb[0])[:, :]
                                  ).then_inc(s_hc, 1)
                for j in range(2):
                    v.wait_ge(s_o, j + 1)
                    v.tensor_copy(ot[j][:, :], (trA if j == 0 else pb[1])[:, :]
                                  ).then_inc(s_oc, 1)

            @block.tensor
            def _(pe):
                pe.wait_ge(s_c, 16 * NCDMA)
                for g in range(GT):
                    w, t = divmod(g, T)
                    pe.wait_ge(s_s, g + 1)
                    pe.wait_ge(s_x[g % XS], 16 * (g // XS + 1))
                    if t == 0 and w >= 2:
                        pe.wait_ge(s_fl, w - 1)
                    pe.matmul(pb[w % 2][:, 0:D + 2], S_sb[g % SS][:, :],
                              xbuf[g % XS][:, :],
                              start=(t == 0), stop=(t == T - 1)).then_inc(s_mm, 1)
                # ---- counts to row + pooled transposes ----
                pe.wait_ge(s_fl, NW)
                for w in range(NW):
                    pe.matmul(trA[0:1, w * 128:(w + 1) * 128], po[w][:, D:D + 1],
                              ident_sb[:, :], start=True, stop=True).then_inc(s_cp, 1)
                for w in range(NW):
                    dst = trB if w < 2 else pb[0]
                    off = (w % 2) * 256
                    for k in range(2):
                        pe.transpose(dst[:, off + k * 128:off + (k + 1) * 128],
                                     po[w][:, k * 128:(k + 1) * 128],
                                     ident_sb[:, :]).then_inc(s_tr, 1)
                # ---- MLP (transposed orientation) ----
                pe.wait_ge(s_ptc, 8)
                pe.wait_ge(s_crc, 1)
                for j in range(2):
                    jc = slice(j * 128, (j + 1) * 128)
                    dst = trA if j == 0 else pb[1]
                    pe.matmul(dst[:, 0:SEG], wink[0][:, jc], pT[0][:, :],
                              start=True, stop=False)
                    pe.matmul(dst[:, 0:SEG], wink[1][:, jc], pT[1][:, :],
                              start=False, stop=False)
                    pe.matmul(dst[:, 0:SEG], bin_sb[0:1, jc], c_row[:, :],
                              start=False, stop=True).then_inc(s_z, 1)
                pe.wait_ge(s_zc, 2)
                for j in range(4):
                    jc = slice(j * 128, (j + 1) * 128)
                    dst = trB if j % 2 == 0 else pb[0]
                    if j >= 2:
                        pe.wait_ge(s_hc, j - 1)
                    pe.matmul(dst[:, 0:SEG], w1k[0][:, jc], zT[0][:, :],
                              start=True, stop=False)
                    pe.matmul(dst[:, 0:SEG], w1k[1][:, jc], zT[1][:, :],
                              start=False, stop=False)
                    pe.matmul(dst[:, 0:SEG], b1_sb[0:1, jc], ones_row[:, :],
                              start=False, stop=True).then_inc(s_h, 1)
                pe.wait_ge(s_hc, 4)
                for j in range(2):
                    jc = slice(j * 128, (j + 1) * 128)
                    dst = trA if j == 0 else pb[1]
                    for i in range(4):
                        pe.matmul(dst[:, 0:SEG], w2k[i][:, jc], hT[i][:, :],
                                  start=(i == 0), stop=False)
                    pe.matmul(dst[:, 0:SEG], b2_sb[0:1, jc], ones_row[:, :],
                              start=False, stop=True).then_inc(s_o, 1)

    return nc


def _prep_inputs(x, batch):
    """Window-aligned shard plan: per core, per window, a tile-aligned row
    range plus localized (window-relative) segment ids."""
    bounds = np.searchsorted(batch, np.arange(0, NSEG + 1, WIN))
    ts = bounds[:-1] // 128
    te = -(-bounds[1:] // 128)
    T = int((te - ts).max())

    iota = np.broadcast_to(np.arange(128, dtype=np.float32), (128, 128)).copy()
    ident = np.eye(128, dtype=np.float32)

    per_core = []
    for c in range(N_CORES):
        m = {}
        for wi in range(NW):
            w = c * NW + wi
            r0 = int(ts[w]) * 128
            r1 = r0 + T * 128
            if r1 <= N:
                xw = x[r0:r1]
                bw = batch[r0:r1]
            else:
                pad = r1 - N
                xw = np.concatenate([x[r0:], np.zeros((pad, D), np.float32)])
                bw = np.concatenate([batch[r0:],
                                     np.full(pad, 10 ** 9, batch.dtype)])
            ba = (bw.astype(np.int64) - w * WIN).astype(np.float32)
            m[f"x{wi}"] = np.ascontiguousarray(xw, dtype=np.float32)
            m[f"ba{wi}"] = np.ascontiguousarray(ba.reshape(T, 128).T)
        m["ones"] = np.ones((128, 2), np.float32)
        m["onesrow"] = np.ones((1, SEG), np.float32)
        m["iota"] = iota
        m["ident"] = ident
        per_core.append(m)
    return T, per_core


def kernel(**inputs):
    x = np.asarray(inputs["x"], dtype=np.float32)
    batch = np.asarray(inputs["batch"])
    W_in = np.ascontiguousarray(np.asarray(inputs["W_in"], np.float32))
    b_in = np.asarray(inputs["b_in"], np.float32).reshape(1, D)
    W1 = np.ascontiguousarray(np.asarray(inputs["W1"], np.float32))
    b1 = np.asarray(inputs["b1"], np.float32).reshape(1, 2 * D)
    W2 = np.ascontiguousarray(np.asarray(inputs["W2"], np.float32))
    b2 = np.asarray(inputs["b2"], np.float32).reshape(1, D)

    T, per_core = _prep_inputs(x, batch)
    for m in per_core:
        m.update(win=W_in, bin=b_in, w1=W1, b1=b1, w2=W2, b2=b2)

    nc = build_program(T)
    res = run_bass_kernel_spmd(nc, per_core, list(range(N_CORES)))

    out = np.empty((NSEG, D), np.float32)
    for c in range(N_CORES):
        out[c * SEG:(c + 1) * SEG, :] = res.results[c]["outT"].T
    return out
